# revision 17
# baseline (speedup 1.0000x reference)
"""Trainium2 Bass kernel for nn_Decoder8to4 (two GRU decoders, B=4096, 32 steps).

Sharding: 8 cores = 2 decoders x 4 batch shards of 1024. SPMD: every core runs
the same program; per-core in_maps carry that core's decoder weights + batch
shard. Everything stays feature-major on chip ([feature, batch]) so the GRU
recurrence needs no transposes.

Per core, per step (B_loc=1024, H=1024, 3H=3072):
  gates g = Waug.T.T @ [h; z; o]  accumulated in PSUM over 11 K-tiles of 128,
  where Waug = [Whh | Wih_z | Wih_o]  (so gi+gh merge for free in PSUM).
  r/z gates: sigmoid straight out of PSUM with per-partition bias (ACT).
  n gate: split A_n = Whh_n @ h (8 K-tiles) and B_n = Wih_n @ [z;o] (3 K-tiles);
  n = tanh(B_n + bih_n + r*(A_n + bhh_n)) via one fused scalar_tensor_tensor.
  h update on DVE (bf16 state); o = Wo.T.T @ h + bo feeds back as next step's
  last K-tile and is staged in SBUF, DMA'd out in contiguous 2MB chunks every
  4 steps. Step 0's SOS one-hot folds into biases (col 127 of Wih_o).

Two scheduling tricks keep the PE at the matmul roofline:
  * the per-core batch is split into two independent 512-wide streams,
    interleaved per H-tile, so each stream's gate-chain / o-feedback latency
    hides under the other stream's matmul block;
  * within each stream the gate chain is software-pipelined one H-tile deep
    (phase A: sigmoids + fused n-gate DVE ops; phase B: tanh + h update) so
    the strict in-order ACT/DVE queues never head-block on each other.
"""

import numpy as np
import ml_dtypes

import concourse.bacc as bacc
import concourse.mybir as mybir
import concourse.tile as tile
from concourse.bass_utils import run_bass_kernel_spmd

BF16 = ml_dtypes.bfloat16

B = 4096
HID = 1024
ZDIM = 256
ODIM = 128
T = 32
N_CORES = 8
BLOC = B // 4          # batch rows per core (4 shards per decoder)
P = 128                # partitions
NKT = 11               # K tiles: 8 h + 2 z + 1 o
KH = HID // P          # 8 h K-tiles
NS = 2                 # independent half-batch streams
SB = BLOC // NS        # 512 batch per stream
TSTAGE = 4             # steps staged in SBUF per output DMA

F32 = mybir.dt.float32
BF = mybir.dt.bfloat16
AF = mybir.ActivationFunctionType
ALU = mybir.AluOpType

# bias column layout inside the packed [128, 65] bias tensor
_BI = 0        # 8 cols: tanh(h0) bias
_BRZ0 = 8      # 16 cols: r/z bias at t=0 (incl. SOS column)
_BRZ = 24      # 16 cols: r/z bias
_BHN = 40      # 8 cols: bhh n-part
_BIN0 = 48     # 8 cols: bih n-part at t=0 (incl. SOS column)
_BIN = 56      # 8 cols: bih n-part
_BO = 64       # 1 col: output bias

# K-tile issue order inside each PSUM group: static z tiles and early-ready h
# tiles first, late h tile (7) and the o feedback tile (10) last.
_JS_RZ0 = [8, 9] + list(range(7)) + [7]            # step 0 (no o tile)
_JS_RZ = [8, 9] + list(range(7)) + [7, 10]         # steps 1..31
_JS_A = list(range(KH))                            # A_n: h tiles only
_JS_B0 = [8, 9]                                    # B_n at step 0
_JS_B = [8, 9, 10]


def build_program(loop_reps=None, dma_mode="sync"):
    nc = bacc.Bacc("TRN2", target_bir_lowering=False, debug=False)

    waug = nc.declare_dram_parameter("waug", [NKT * P, 3 * HID], BF, isOutput=False)
    wi = nc.declare_dram_parameter("wi", [ZDIM, HID], BF, isOutput=False)
    wo = nc.declare_dram_parameter("wo", [HID, ODIM], BF, isOutput=False)
    zin = nc.declare_dram_parameter("z", [ZDIM, BLOC], BF, isOutput=False)
    biases = nc.declare_dram_parameter("biases", [P, 65], F32, isOutput=False)
    out = nc.declare_dram_parameter(
        "out", [T // TSTAGE, ODIM, TSTAGE * BLOC], F32, isOutput=True
    )

    with tile.TileContext(nc) as tc:
        import contextlib

        with contextlib.ExitStack() as ctx:
            wpool = ctx.enter_context(tc.tile_pool(name="w", bufs=1))
            state = ctx.enter_context(tc.tile_pool(name="state", bufs=1))
            dbuf = ctx.enter_context(tc.tile_pool(name="dbuf", bufs=2))
            tmp = ctx.enter_context(tc.tile_pool(name="tmp", bufs=2))
            psum = ctx.enter_context(tc.tile_pool(name="ps", bufs=1, space="PSUM"))

            wa = []
            for j in range(NKT):
                t_ = wpool.tile([P, 3 * HID], BF, tag=f"wa{j}", name=f"wa{j}")
                nc.sync.dma_start(t_[:], waug[j * P : (j + 1) * P, :])
                wa.append(t_)
            wit = []
            for j in range(ZDIM // P):
                t_ = wpool.tile([P, HID], BF, tag=f"wi{j}", name=f"wi{j}")
                nc.sync.dma_start(t_[:], wi[j * P : (j + 1) * P, :])
                wit.append(t_)
            wot = []
            for j in range(KH):
                t_ = wpool.tile([P, ODIM], BF, tag=f"wo{j}", name=f"wo{j}")
                nc.sync.dma_start(t_[:], wo[j * P : (j + 1) * P, :])
                wot.append(t_)
            bias = wpool.tile([P, 65], F32, tag="bias", name="bias")
            nc.sync.dma_start(bias[:], biases[:])

            def bcol(c):
                return bias[:, c : c + 1]

            zb = []
            for j in range(ZDIM // P):
                t_ = state.tile([P, BLOC], BF, tag=f"zb{j}", name=f"zb{j}")
                nc.sync.dma_start(t_[:], zin[j * P : (j + 1) * P, :])
                zb.append(t_)

            loop_cm = (
                tc.For_i(0, loop_reps, 1) if loop_reps else contextlib.nullcontext()
            )
            ctx.enter_context(loop_cm)

            # per-stream state: hb[s][k] bf16 [P, SB]
            hb = [[None] * KH for _ in range(NS)]
            ob = [None] * NS
            stage = None

            def ssl(s):
                return slice(s * SB, (s + 1) * SB)

            # ---- h0 per stream ----
            for k in range(KH):
                for s in range(NS):
                    ph = psum.tile([P, SB], F32, tag=f"pa{s}", name="ph")
                    for j in range(ZDIM // P):
                        nc.tensor.matmul(
                            ph[:],
                            wit[j][:, k * P : (k + 1) * P],
                            zb[j][:, ssl(s)],
                            start=(j == 0),
                            stop=(j == ZDIM // P - 1),
                        )
                    hb[s][k] = dbuf.tile(
                        [P, SB], BF, tag=f"hb{s}_{k}", name=f"hb{s}_{k}"
                    )
                    nc.scalar.activation(
                        hb[s][k][:], ph[:], AF.Tanh, bias=bcol(_BI + k)
                    )

            def emit_A(t, s, k, hb_cur):
                first = t == 0
                js_rz = _JS_RZ0 if first else _JS_RZ
                js_b = _JS_B0 if first else _JS_B
                brz = _BRZ0 if first else _BRZ

                def rhs(j):
                    if j < KH:
                        return hb_cur[s][j][:]
                    if j < KH + 2:
                        return zb[j - KH][:, ssl(s)]
                    return ob[s][:]

                pg = {}
                for gate, m, js in (
                    ("r", k, js_rz),
                    ("z", KH + k, js_rz),
                    ("a", 2 * KH + k, _JS_A),
                    ("b", 2 * KH + k, js_b),
                ):
                    pg[gate] = psum.tile(
                        [P, SB], F32, tag=f"p{gate}{s}", name=f"p{gate}{s}"
                    )
                    for j in js:
                        nc.tensor.matmul(
                            pg[gate][:],
                            wa[j][:, m * P : (m + 1) * P],
                            rhs(j),
                            start=(j == js[0]),
                            stop=(j == js[-1]),
                        )
                rt = tmp.tile([P, SB], F32, tag=f"rt{s}", name=f"rt{s}")
                zt = tmp.tile([P, SB], F32, tag=f"zt{s}", name=f"zt{s}")
                nc.scalar.activation(rt[:], pg["r"][:], AF.Sigmoid, bias=bcol(brz + k))
                nc.scalar.activation(
                    zt[:], pg["z"][:], AF.Sigmoid, bias=bcol(brz + KH + k)
                )
                t1 = tmp.tile([P, SB], F32, tag=f"t1{s}", name=f"t1{s}")
                nc.vector.scalar_tensor_tensor(
                    t1[:], pg["a"][:], bcol(_BHN + k), rt[:], op0=ALU.add, op1=ALU.mult
                )
                nc.vector.tensor_add(t1[:], t1[:], pg["b"][:])
                return zt, t1

            def emit_B(t, s, k, zt, t1, hb_old):
                bin_ = _BIN0 if t == 0 else _BIN
                nt = tmp.tile([P, SB], F32, tag=f"nt{s}", name=f"nt{s}")
                nc.scalar.activation(nt[:], t1[:], AF.Tanh, bias=bcol(bin_ + k))
                dt_ = tmp.tile([P, SB], F32, tag=f"dt{s}", name=f"dt{s}")
                nc.vector.scalar_tensor_tensor(
                    dt_[:], nt[:], -1.0, hb_old[:], op0=ALU.mult, op1=ALU.add
                )
                nc.vector.tensor_mul(dt_[:], zt[:], dt_[:])
                hnew = dbuf.tile([P, SB], BF, tag=f"hb{s}_{k}", name=f"hb{s}_{k}")
                nc.vector.tensor_add(hnew[:], nt[:], dt_[:])
                return hnew

            for t in range(T):
                hb_old = [list(hb[s]) for s in range(NS)]
                hb_new = [[None] * KH for _ in range(NS)]
                pend = [None] * NS
                for k in range(KH + 1):
                    for s in range(NS):
                        if k < KH:
                            zt, t1 = emit_A(t, s, k, hb_old)
                            nxt = (k, zt, t1)
                        else:
                            nxt = None
                        if pend[s] is not None:
                            pk, pzt, pt1 = pend[s]
                            hb_new[s][pk] = emit_B(t, s, pk, pzt, pt1, hb_old[s][pk])
                        pend[s] = nxt
                hb = hb_new

                if t % TSTAGE == 0:
                    stage = tmp.tile(
                        [P, TSTAGE * BLOC], F32, tag="stage", name="stage",
                        bufs=2 if TSTAGE <= 4 else 1,
                    )
                so = (t % TSTAGE) * BLOC
                for s in range(NS):
                    po = psum.tile([P, SB], F32, tag=f"pz{s}", name=f"po{s}")
                    for j in range(KH):
                        nc.tensor.matmul(
                            po[:],
                            wot[j][:],
                            hb[s][j][:],
                            start=(j == 0),
                            stop=(j == KH - 1),
                        )
                    nc.scalar.activation(
                        stage[:, so + s * SB : so + (s + 1) * SB],
                        po[:],
                        AF.Identity,
                        bias=bcol(_BO),
                    )
                    ob[s] = dbuf.tile([P, SB], BF, tag=f"ob{s}", name=f"ob{s}")
                    nc.vector.tensor_copy(
                        ob[s][:], stage[:, so + s * SB : so + (s + 1) * SB]
                    )
                if t % TSTAGE == TSTAGE - 1 and dma_mode != "none":
                    eng = nc.gpsimd if dma_mode == "gpsimd" else nc.sync
                    eng.dma_start(out[t // TSTAGE, :, :], stage[:])

    nc.compile()
    return nc


def prep_core_inputs(inputs, core, _cache={}):
    d, q = divmod(core, 4)
    sfx = str(d)
    z = np.asarray(inputs["z_8p" if d == 0 else "z_8r"], np.float32)
    if d not in _cache:
        Wi = np.asarray(inputs["Wi" + sfx], np.float32)
        bi = np.asarray(inputs["bi" + sfx], np.float32)
        Wih = np.asarray(inputs["Wih" + sfx], np.float32)
        Whh = np.asarray(inputs["Whh" + sfx], np.float32)
        bih = np.asarray(inputs["bih" + sfx], np.float32)
        bhh = np.asarray(inputs["bhh" + sfx], np.float32)
        Wo = np.asarray(inputs["Wo" + sfx], np.float32)
        bo = np.asarray(inputs["bo" + sfx], np.float32)

        waug = np.ascontiguousarray(
            np.concatenate([Whh, Wih[:, ODIM:], Wih[:, :ODIM]], axis=1).T
        ).astype(BF16)
        sos = Wih[:, ODIM - 1]  # SOS one-hot contribution
        brzsum = bih[: 2 * HID] + bhh[: 2 * HID]
        cols = [
            bi.reshape(KH, P).T,                                   # _BI
            (brzsum + sos[: 2 * HID]).reshape(16, P).T,            # _BRZ0
            brzsum.reshape(16, P).T,                               # _BRZ
            bhh[2 * HID :].reshape(KH, P).T,                       # _BHN
            (bih[2 * HID :] + sos[2 * HID :]).reshape(KH, P).T,    # _BIN0
            bih[2 * HID :].reshape(KH, P).T,                       # _BIN
            bo.reshape(1, P).T,                                    # _BO
        ]
        _cache[d] = {
            "waug": waug,
            "wi": np.ascontiguousarray(Wi.T).astype(BF16),
            "wo": np.ascontiguousarray(Wo.T).astype(BF16),
            "biases": np.ascontiguousarray(np.concatenate(cols, axis=1), np.float32),
        }
    zt = np.ascontiguousarray(z[q * BLOC : (q + 1) * BLOC].T).astype(BF16)
    return dict(_cache[d], z=zt)


_NC_CACHE = None


def get_program():
    global _NC_CACHE
    if _NC_CACHE is None:
        _NC_CACHE = build_program()
    return _NC_CACHE


def run(inputs, **run_kwargs):
    nc = get_program()
    in_maps = [prep_core_inputs(inputs, c) for c in range(N_CORES)]
    res = run_bass_kernel_spmd(nc, in_maps, list(range(N_CORES)), **run_kwargs)
    outs = []
    for d in range(2):
        parts = []
        for q in range(4):
            o = res.results[d * 4 + q]["out"]  # [T/TS, ODIM, TS*BLOC]
            o = (
                o.reshape(T // TSTAGE, ODIM, TSTAGE, BLOC)
                .transpose(0, 2, 3, 1)
                .reshape(T, BLOC, ODIM)
                .transpose(1, 0, 2)
            )
            parts.append(np.ascontiguousarray(o))
        outs.append(np.concatenate(parts, axis=0))
    return (outs[0], outs[1]), res


def kernel(**inputs):
    (z4p, z4r), _ = run(inputs)
    return z4p, z4r
